# revision 13
# baseline (speedup 1.0000x reference)
"""Trainium2 Bass kernel for cross "efficient attention".

Reference computation (per batch b, head h, with C=128, HEADS=8, hc=16, n=16384):
    k = x2[b].reshape(HEADS, hc, n); v = x1[b].reshape(HEADS, hc, n)
    key_sm   = softmax(k, axis=-1)          # over n
    query_sm = softmax(k, axis=1)           # over hc (head channels)
    context  = key_sm @ v^T                 # (hc, hc)
    out[b,h] = context^T @ query_sm         # (hc, n)

Sharding: data-parallel over batch B=8 across the 8 NeuronCores (no
collectives).  Inputs are cast to bf16 on the host (tolerance is 2e-2;
measured ~6e-3 end to end) and packed per slab as [x2_slab | x1t_slab]
into ONE dram stream so each slab is a single large contiguous DMA.

Key structure (vs the previous revision):
  * The query-softmax normalization (divide by cs) moved to the HOST:
    the kernel ships raw att = bd^T @ e plus cs (bf16, 256 KiB) and the
    host divides.  This removes the broadcast tensor_mul on DVE that
    previously paced pass 2 at ~2x the store rate.
  * Pass-2 evictions (f32 PSUM att -> bf16 SBUF) alternate between DVE
    and the ACT engine (idle after the exps), so stores are DMA-paced.
Pass 1 per slab: one xin DMA -> exp (ACT, rowsum accum) -> per-chunk
transpose matmul + 8-col colsum matmul -> one wide DVE copy per
8-chunk group -> ctx matmuls (lagging one group) -> per-slab cs
eviction (ACT copy, f32 PSUM -> bf16 cs_all).
bd = (ctx / rowsum) * blockdiag, then pass 2 per 2048 block: 16 att
matmuls -> evict (DVE/ACT) -> store on the SP ring.
Output leaves the device transposed ([128, N/128, C] blocks); the host
reassembles [C, H, W] and applies the cs division.
"""

import numpy as np
from contextlib import ExitStack

B, C, H, W = 8, 128, 128, 128
N = H * W                 # 16384
HEADS, HC = 8, 16
NCORES = 8
SLABS = [1024, 2048, 4096, 4096, 2048, 1024, 1024, 512, 512]
NSLAB = len(SLABS)
assert sum(SLABS) == N
NB = N // C               # 128 chunk-blocks total
GRP = 8                   # transpose chunks batched per PSUM group tile
OB = 2048                 # pass-2 output block width
NOB = N // OB             # 8
OCH = OB // C             # chunks per output block = 16

_cache: dict = {}


def _build():
    import concourse.bass as bass
    import concourse.tile as tile
    from concourse import bacc, mybir

    FP32 = mybir.dt.float32
    BF16 = mybir.dt.bfloat16
    AF = mybir.ActivationFunctionType

    nc = bacc.Bacc("TRN2", target_bir_lowering=False, debug=False)

    xin_d = nc.dram_tensor("xin", [C, 2 * N], BF16, kind="ExternalInput")
    id_d = nc.dram_tensor("ident", [C, C], BF16, kind="ExternalInput")
    ind8_d = nc.dram_tensor("ind8", [C, HEADS], BF16, kind="ExternalInput")
    bd8_d = nc.dram_tensor("bd8", [C, C], BF16, kind="ExternalInput")
    out_d = nc.dram_tensor("out", [C, NB, C], BF16, kind="ExternalOutput")
    cs_d = nc.dram_tensor("cs", [C, NB * HEADS], BF16, kind="ExternalOutput")

    with tile.TileContext(nc) as tc:
        with ExitStack() as ctx:
            persist = ctx.enter_context(tc.tile_pool(name="persist", bufs=1))
            xinp = ctx.enter_context(tc.tile_pool(name="xinp", bufs=3))
            eTp = ctx.enter_context(tc.tile_pool(name="eTp", bufs=3))
            outp = ctx.enter_context(tc.tile_pool(name="outp", bufs=4))
            smalls = ctx.enter_context(tc.tile_pool(name="smalls", bufs=1))

            # one exp tile per slab (a single big tile would serialize on
            # tile-granular write-after-read hazards)
            exp_tiles = [
                persist.tile([C, SW], BF16, tag=f"exp{i}", name=f"exp{i}")
                for i, SW in enumerate(SLABS)
            ]
            cs_all = persist.tile([C, NB * HEADS], BF16, tag="cs_all")
            rs_acc = smalls.tile([C, NSLAB], FP32, tag="rs_acc")
            ident = smalls.tile([C, C], BF16, tag="ident")
            ind8 = smalls.tile([C, HEADS], BF16, tag="ind8")
            bd8 = smalls.tile([C, C], BF16, tag="bd8")
            bd = smalls.tile([C, C], BF16, tag="bd")

            with tc.tile_pool(name="psctx", bufs=1, space="PSUM") as ps_ctx, \
                 tc.tile_pool(name="pstre", bufs=3, space="PSUM") as ps_te, \
                 tc.tile_pool(name="pscs", bufs=2, space="PSUM") as ps_cs:
                ctx_ps = ps_ctx.tile([C, C], FP32, tag="ctx")

                mm_idx = 0
                pending = []   # (eT_ap, vT_ap) per not-yet-contracted chunk

                def emit_ctx(k):
                    nonlocal mm_idx
                    for eTc, vTc in pending[:k]:
                        nc.tensor.matmul(
                            ctx_ps[:], eTc, vTc,
                            start=(mm_idx == 0),
                            stop=(mm_idx == NB - 1),
                        )
                        mm_idx += 1
                    del pending[:k]

                off = 0
                chunk_aps = []   # global chunk index -> exp chunk AP
                for i, SW in enumerate(SLABS):
                    nch = SW // C
                    ngrp = (nch + GRP - 1) // GRP
                    xt = xinp.tile([C, 2 * SW], BF16, tag="xt")
                    # slab 0 rides SWDGE: the gpsimd queue exits the start
                    # preamble ~3us before the SP HWDGE ring
                    ldeng = nc.gpsimd if i == 0 else nc.sync
                    ldeng.dma_start(
                        out=xt[:], in_=xin_d[:, bass.ds(2 * off, 2 * SW)]
                    )
                    if i == 0:
                        # constants ride the ACT ring so they don't delay
                        # the slab loads on the SP ring
                        nc.scalar.dma_start(out=ident[:], in_=id_d[:])
                        nc.scalar.dma_start(out=ind8[:], in_=ind8_d[:])
                        nc.scalar.dma_start(out=bd8[:], in_=bd8_d[:])

                    exp_sl = exp_tiles[i]
                    nc.scalar.activation(
                        exp_sl[:], xt[:, bass.ds(0, SW)], AF.Exp,
                        accum_out=rs_acc[:, i:i + 1],
                    )

                    vTv = xt[:, bass.ds(SW, SW)].rearrange(
                        "p (j c) -> p j c", c=C
                    )
                    eT = eTp.tile([C, nch * C], BF16, tag="eT")
                    eTv = eT[:].rearrange("p (j c) -> p j c", c=C)
                    cs_ps = ps_cs.tile([C, nch * HEADS], FP32, tag="cs")
                    for g in range(ngrp):
                        gsz = min(GRP, nch - g * GRP)
                        te = ps_te.tile([C, gsz * C], BF16, tag="te")
                        fresh = []
                        for jj in range(gsz):
                            j = g * GRP + jj
                            e_chunk = exp_sl[:, bass.ds(j * C, C)]
                            chunk_aps.append(e_chunk)
                            nc.tensor.transpose(
                                te[:, bass.ds(jj * C, C)], e_chunk, ident[:]
                            )
                            nc.tensor.matmul(
                                cs_ps[:, bass.ds(j * HEADS, HEADS)],
                                e_chunk, ind8[:],
                            )
                            fresh.append((eTv[:, j, :], vTv[:, j, :]))
                        nc.vector.tensor_copy(
                            eT[:, bass.ds(g * GRP * C, gsz * C)], te[:]
                        )
                        # ctx matmuls lag one group behind the copies
                        emit_ctx(len(pending))
                        pending.extend(fresh)

                    # evict this slab's colsums (ACT engine; DVE is busy
                    # with the te copies)
                    nc.scalar.copy(
                        cs_all[:, bass.ds(off // C * HEADS, nch * HEADS)],
                        cs_ps[:],
                    )
                    off += SW
                emit_ctx(len(pending))

                # ---- block-diagonal context weights ----
                rowsum = smalls.tile([C, 1], FP32, tag="rowsum")
                nc.vector.tensor_reduce(
                    rowsum[:], rs_acc[:], mybir.AxisListType.X, mybir.AluOpType.add
                )
                rs_rcp = smalls.tile([C, 1], FP32, tag="rs_rcp")
                nc.vector.reciprocal(rs_rcp[:], rowsum[:])
                scaled = smalls.tile([C, C], BF16, tag="scaled")
                nc.vector.tensor_scalar(
                    scaled[:], ctx_ps[:], rs_rcp[:, 0:1], None, mybir.AluOpType.mult
                )
                nc.vector.tensor_mul(bd[:], scaled[:], bd8[:])

            # cs ships to the host (256 KiB) on the idle gpsimd ring,
            # overlapping pass 2
            nc.gpsimd.dma_start(out=cs_d[:], in_=cs_all[:])

            # ---- pass 2: raw attended (transposed), store ----
            with tc.tile_pool(name="psatt", bufs=2, space="PSUM") as ps_att:
                for b in range(NOB):
                    att = ps_att.tile([C, OB], FP32, tag="att")
                    for j in range(OCH):
                        nc.tensor.matmul(
                            att[:, bass.ds(j * C, C)],
                            chunk_aps[b * OCH + j],
                            bd[:],
                        )
                    ot = outp.tile([C, OB], BF16, tag="ot")
                    # evictions alternate DVE (2x bf16) / ACT (idle)
                    if b % 2 == 0:
                        nc.vector.tensor_copy(ot[:], att[:])
                    else:
                        nc.scalar.copy(ot[:], att[:])
                    nc.sync.dma_start(
                        out=out_d[:, bass.ds(b * OCH, OCH), :],
                        in_=ot[:].rearrange("p (j c) -> p j c", c=C),
                    )

    nc.compile()
    return nc


def _get_nc():
    if "nc" not in _cache:
        _cache["nc"] = _build()
    return _cache["nc"]


def _consts_np():
    import ml_dtypes

    bf16 = ml_dtypes.bfloat16
    ident = np.eye(C, dtype=np.float32).astype(bf16)
    ind8 = np.zeros((C, HEADS), dtype=np.float32)
    for h in range(HEADS):
        ind8[h * HC:(h + 1) * HC, h] = 1.0
    bd8 = np.zeros((C, C), dtype=np.float32)
    for h in range(HEADS):
        bd8[h * HC:(h + 1) * HC, h * HC:(h + 1) * HC] = 1.0
    return ident, ind8.astype(bf16), bd8.astype(bf16)


def _to_np(a) -> np.ndarray:
    """Materialize to float32 numpy; retry once on a transient bad fetch
    (device-backed arrays have been observed to materialize NaNs once)."""
    out = np.asarray(a, dtype=np.float32)
    if np.isnan(out).any():
        out = np.asarray(a, dtype=np.float32)
    return out


def make_in_maps(x1: np.ndarray, x2: np.ndarray):
    import ml_dtypes

    bf16 = ml_dtypes.bfloat16
    x1 = _to_np(x1).reshape(B, C, N)
    x2 = _to_np(x2).reshape(B, C, N)
    # x1 blocked-transposed: x1t[b, p, j, c] = x1[b, c, j*128 + p]
    x1t = np.ascontiguousarray(
        x1.reshape(B, C, NB, C).transpose(0, 3, 2, 1)
    ).reshape(B, C, N)
    # interleave per slab: [x2_slab | x1t_slab]
    xin = np.empty((B, C, 2 * N), dtype=np.float32)
    off = 0
    for SW in SLABS:
        xin[:, :, 2 * off:2 * off + SW] = x2[:, :, off:off + SW]
        xin[:, :, 2 * off + SW:2 * off + 2 * SW] = x1t[:, :, off:off + SW]
        off += SW
    xin = xin.astype(bf16)
    ident, ind8, bd8 = _consts_np()
    return [
        {"xin": xin[i], "ident": ident, "ind8": ind8, "bd8": bd8}
        for i in range(NCORES)
    ]


def kernel(x1: np.ndarray, x2: np.ndarray) -> np.ndarray:
    from concourse.bass_utils import run_bass_kernel_spmd

    nc = _get_nc()
    in_maps = make_in_maps(x1, x2)
    res = run_bass_kernel_spmd(nc, in_maps, core_ids=list(range(NCORES)))
    outs = []
    for i in range(NCORES):
        o = np.asarray(res.results[i]["out"], dtype=np.float32)  # [128, NB, C]
        cs = np.asarray(res.results[i]["cs"], dtype=np.float32)  # [128, NB*8]
        att = o.transpose(2, 1, 0).reshape(C, N)                 # [C, N] raw
        cs_t = cs.reshape(C, NB, HEADS).transpose(2, 1, 0).reshape(HEADS, N)
        outs.append(att.reshape(HEADS, HC, N) / cs_t[:, None, :])
    return np.stack(outs, axis=0).reshape(B, C, H, W)


# revision 17
# speedup vs baseline: 1.2411x; 1.2411x over previous
"""Trainium2 Bass kernel for cross "efficient attention".

Reference computation (per batch b, head h, with C=128, HEADS=8, hc=16, n=16384):
    k = x2[b].reshape(HEADS, hc, n); v = x1[b].reshape(HEADS, hc, n)
    key_sm   = softmax(k, axis=-1)          # over n
    query_sm = softmax(k, axis=1)           # over hc (head channels)
    context  = key_sm @ v^T                 # (hc, hc)
    out[b,h] = context^T @ query_sm         # (hc, n)

Sharding: data-parallel over batch B=8 across the 8 NeuronCores (no
collectives).  Inputs are cast to bf16 on the host (tolerance is 2e-2;
measured ~6e-3 end to end) and packed per slab as [x2_slab | x1t_slab]
into ONE dram stream so each slab is a single large contiguous DMA.

Key structure (vs the previous revision):
  * The query-softmax normalization (divide by cs) moved to the HOST:
    the kernel ships raw att = bd^T @ e plus cs (bf16, 256 KiB) and the
    host divides.  This removes the broadcast tensor_mul on DVE that
    previously paced pass 2 at ~2x the store rate.
  * Pass-2 evictions (f32 PSUM att -> bf16 SBUF) alternate between DVE
    and the ACT engine (idle after the exps), so stores are DMA-paced.
Pass 1 per slab: one xin DMA -> exp (ACT, rowsum accum) -> per-chunk
transpose matmul + 8-col colsum matmul -> one wide DVE copy per
8-chunk group -> ctx matmuls (lagging one group) -> per-slab cs
eviction (ACT copy, f32 PSUM -> bf16 cs_all).
bd = (ctx / rowsum) * blockdiag, then pass 2 per 2048 block: 16 att
matmuls -> evict (DVE/ACT) -> store on the SP ring.
Output leaves the device transposed ([128, N/128, C] blocks); the host
reassembles [C, H, W] and applies the cs division.
"""

import numpy as np
from contextlib import ExitStack

B, C, H, W = 8, 128, 128, 128
N = H * W                 # 16384
HEADS, HC = 8, 16
NCORES = 8
# small first slab so the first exp starts early; small last slab so the
# ctx -> bd tail after the final load is short; uniform middles sized so
# every slab gets a dedicated SBUF buffer (no write-after-read stalls)
SLABS = [512, 1024] + [2048] * 7 + [512]
NSLAB = len(SLABS)
assert sum(SLABS) == N
NB = N // C               # 128 chunk-blocks total
GRP = 8                   # transpose chunks batched per PSUM group tile
OB = 2048                 # pass-2 output block width
NOB = N // OB             # 8
OCH = OB // C             # chunks per output block = 16

_cache: dict = {}


def _build():
    import concourse.bass as bass
    import concourse.tile as tile
    from concourse import bacc, mybir

    FP32 = mybir.dt.float32
    BF16 = mybir.dt.bfloat16
    AF = mybir.ActivationFunctionType

    nc = bacc.Bacc("TRN2", target_bir_lowering=False, debug=False)

    xin_d = nc.dram_tensor("xin", [C, 2 * N], BF16, kind="ExternalInput")
    id_d = nc.dram_tensor("ident", [C, C], BF16, kind="ExternalInput")
    ind8_d = nc.dram_tensor("ind8", [C, HEADS], BF16, kind="ExternalInput")
    bd8_d = nc.dram_tensor("bd8", [C, C], BF16, kind="ExternalInput")
    out_d = nc.dram_tensor("out", [C, NB, C], BF16, kind="ExternalOutput")
    cs_d = nc.dram_tensor("cs", [C, NB * HEADS], BF16, kind="ExternalOutput")

    with tile.TileContext(nc) as tc:
        with ExitStack() as ctx:
            persist = ctx.enter_context(tc.tile_pool(name="persist", bufs=1))
            xinp = ctx.enter_context(tc.tile_pool(name="xinp", bufs=NSLAB))
            eTp = ctx.enter_context(tc.tile_pool(name="eTp", bufs=3))
            outp = ctx.enter_context(tc.tile_pool(name="outp", bufs=4))
            smalls = ctx.enter_context(tc.tile_pool(name="smalls", bufs=1))

            # one exp tile per slab (a single big tile would serialize on
            # tile-granular write-after-read hazards)
            exp_tiles = [
                persist.tile([C, SW], BF16, tag=f"exp{i}", name=f"exp{i}")
                for i, SW in enumerate(SLABS)
            ]
            cs_all = persist.tile([C, NB * HEADS], BF16, tag="cs_all")
            rs_acc = smalls.tile([C, NSLAB], FP32, tag="rs_acc")
            ident = smalls.tile([C, C], BF16, tag="ident")
            ind8 = smalls.tile([C, HEADS], BF16, tag="ind8")
            bd8 = smalls.tile([C, C], BF16, tag="bd8")
            bd = smalls.tile([C, C], BF16, tag="bd")

            with tc.tile_pool(name="psctx", bufs=1, space="PSUM") as ps_ctx, \
                 tc.tile_pool(name="pstre", bufs=3, space="PSUM") as ps_te, \
                 tc.tile_pool(name="pscs", bufs=2, space="PSUM") as ps_cs:
                ctx_ps = ps_ctx.tile([C, C], FP32, tag="ctx")

                mm_idx = 0
                pending = []   # (eT_ap, vT_ap) per not-yet-contracted chunk

                def emit_ctx(k):
                    nonlocal mm_idx
                    for eTc, vTc in pending[:k]:
                        nc.tensor.matmul(
                            ctx_ps[:], eTc, vTc,
                            start=(mm_idx == 0),
                            stop=(mm_idx == NB - 1),
                        )
                        mm_idx += 1
                    del pending[:k]

                off = 0
                chunk_aps = []   # global chunk index -> exp chunk AP
                for i, SW in enumerate(SLABS):
                    nch = SW // C
                    ngrp = (nch + GRP - 1) // GRP
                    xt = xinp.tile([C, 2 * SW], BF16, tag="xt")
                    if i == 0:
                        # slab 0's x2 half rides SWDGE so exp0 can start as
                        # soon as the engines come up; a big SWDGE transfer
                        # would finish LAST under round-robin contention with
                        # the sync-ring loads, so keep it small and put the
                        # v half on the sync ring instead.
                        nc.gpsimd.dma_start(
                            out=xt[:, bass.ds(0, SW)],
                            in_=xin_d[:, bass.ds(2 * off, SW)],
                        )
                        # ident leads the sync ring (first transposes need it)
                        nc.sync.dma_start(out=ident[:], in_=id_d[:])
                        nc.sync.dma_start(
                            out=xt[:, bass.ds(SW, SW)],
                            in_=xin_d[:, bass.ds(2 * off + SW, SW)],
                        )
                        # remaining constants ride the otherwise-idle ACT ring
                        nc.scalar.dma_start(out=ind8[:], in_=ind8_d[:])
                        nc.scalar.dma_start(out=bd8[:], in_=bd8_d[:])
                    else:
                        nc.sync.dma_start(
                            out=xt[:], in_=xin_d[:, bass.ds(2 * off, 2 * SW)]
                        )

                    exp_sl = exp_tiles[i]
                    nc.scalar.activation(
                        exp_sl[:], xt[:, bass.ds(0, SW)], AF.Exp,
                        accum_out=rs_acc[:, i:i + 1],
                    )

                    vTv = xt[:, bass.ds(SW, SW)].rearrange(
                        "p (j c) -> p j c", c=C
                    )
                    eT = eTp.tile([C, nch * C], BF16, tag="eT")
                    eTv = eT[:].rearrange("p (j c) -> p j c", c=C)
                    cs_ps = ps_cs.tile([C, nch * HEADS], FP32, tag="cs")
                    for g in range(ngrp):
                        gsz = min(GRP, nch - g * GRP)
                        te = ps_te.tile([C, gsz * C], BF16, tag="te")
                        fresh = []
                        for jj in range(gsz):
                            j = g * GRP + jj
                            e_chunk = exp_sl[:, bass.ds(j * C, C)]
                            chunk_aps.append(e_chunk)
                            nc.tensor.transpose(
                                te[:, bass.ds(jj * C, C)], e_chunk, ident[:]
                            )
                            nc.tensor.matmul(
                                cs_ps[:, bass.ds(j * HEADS, HEADS)],
                                e_chunk, ind8[:],
                            )
                            fresh.append((eTv[:, j, :], vTv[:, j, :]))
                        nc.vector.tensor_copy(
                            eT[:, bass.ds(g * GRP * C, gsz * C)], te[:]
                        )
                        # ctx matmuls lag one group behind the copies
                        emit_ctx(len(pending))
                        pending.extend(fresh)

                    # evict this slab's colsums (ACT engine; DVE is busy
                    # with the te copies)
                    nc.scalar.copy(
                        cs_all[:, bass.ds(off // C * HEADS, nch * HEADS)],
                        cs_ps[:],
                    )
                    off += SW
                emit_ctx(len(pending))

                # ---- block-diagonal context weights ----
                rowsum = smalls.tile([C, 1], FP32, tag="rowsum")
                nc.vector.tensor_reduce(
                    rowsum[:], rs_acc[:], mybir.AxisListType.X, mybir.AluOpType.add
                )
                rs_rcp = smalls.tile([C, 1], FP32, tag="rs_rcp")
                nc.vector.reciprocal(rs_rcp[:], rowsum[:])
                scaled = smalls.tile([C, C], BF16, tag="scaled")
                nc.vector.tensor_scalar(
                    scaled[:], ctx_ps[:], rs_rcp[:, 0:1], None, mybir.AluOpType.mult
                )
                nc.vector.tensor_mul(bd[:], scaled[:], bd8[:])

            # cs ships to the host (256 KiB) on the idle gpsimd ring,
            # overlapping pass 2
            nc.gpsimd.dma_start(out=cs_d[:], in_=cs_all[:])

            # ---- pass 2: raw attended (transposed), store ----
            with tc.tile_pool(name="psatt", bufs=2, space="PSUM") as ps_att:
                for b in range(NOB):
                    att = ps_att.tile([C, OB], FP32, tag="att")
                    for j in range(OCH):
                        nc.tensor.matmul(
                            att[:, bass.ds(j * C, C)],
                            chunk_aps[b * OCH + j],
                            bd[:],
                        )
                    ot = outp.tile([C, OB], BF16, tag="ot")
                    # evict each block in halves, DVE + ACT concurrently
                    hb = OB // 2
                    nc.vector.tensor_copy(
                        ot[:, bass.ds(0, hb)], att[:, bass.ds(0, hb)]
                    )
                    nc.scalar.copy(
                        ot[:, bass.ds(hb, hb)], att[:, bass.ds(hb, hb)]
                    )
                    nc.sync.dma_start(
                        out=out_d[:, bass.ds(b * OCH, OCH), :],
                        in_=ot[:].rearrange("p (j c) -> p j c", c=C),
                    )

    nc.compile()
    return nc


def _get_nc():
    if "nc" not in _cache:
        _cache["nc"] = _build()
    return _cache["nc"]


def _consts_np():
    import ml_dtypes

    bf16 = ml_dtypes.bfloat16
    ident = np.eye(C, dtype=np.float32).astype(bf16)
    ind8 = np.zeros((C, HEADS), dtype=np.float32)
    for h in range(HEADS):
        ind8[h * HC:(h + 1) * HC, h] = 1.0
    bd8 = np.zeros((C, C), dtype=np.float32)
    for h in range(HEADS):
        bd8[h * HC:(h + 1) * HC, h * HC:(h + 1) * HC] = 1.0
    return ident, ind8.astype(bf16), bd8.astype(bf16)


def _to_np(a) -> np.ndarray:
    """Materialize to float32 numpy; retry once on a transient bad fetch
    (device-backed arrays have been observed to materialize NaNs once)."""
    out = np.asarray(a, dtype=np.float32)
    if np.isnan(out).any():
        out = np.asarray(a, dtype=np.float32)
    return out


def make_in_maps(x1: np.ndarray, x2: np.ndarray):
    import ml_dtypes

    bf16 = ml_dtypes.bfloat16
    x1 = _to_np(x1).reshape(B, C, N)
    x2 = _to_np(x2).reshape(B, C, N)
    # x1 blocked-transposed: x1t[b, p, j, c] = x1[b, c, j*128 + p]
    x1t = np.ascontiguousarray(
        x1.reshape(B, C, NB, C).transpose(0, 3, 2, 1)
    ).reshape(B, C, N)
    # interleave per slab: [x2_slab | x1t_slab]
    xin = np.empty((B, C, 2 * N), dtype=np.float32)
    off = 0
    for SW in SLABS:
        xin[:, :, 2 * off:2 * off + SW] = x2[:, :, off:off + SW]
        xin[:, :, 2 * off + SW:2 * off + 2 * SW] = x1t[:, :, off:off + SW]
        off += SW
    xin = xin.astype(bf16)
    ident, ind8, bd8 = _consts_np()
    return [
        {"xin": xin[i], "ident": ident, "ind8": ind8, "bd8": bd8}
        for i in range(NCORES)
    ]


def kernel(x1: np.ndarray, x2: np.ndarray) -> np.ndarray:
    from concourse.bass_utils import run_bass_kernel_spmd

    nc = _get_nc()
    in_maps = make_in_maps(x1, x2)
    res = run_bass_kernel_spmd(nc, in_maps, core_ids=list(range(NCORES)))
    outs = []
    for i in range(NCORES):
        o = np.asarray(res.results[i]["out"], dtype=np.float32)  # [128, NB, C]
        cs = np.asarray(res.results[i]["cs"], dtype=np.float32)  # [128, NB*8]
        att = o.transpose(2, 1, 0).reshape(C, N)                 # [C, N] raw
        cs_t = cs.reshape(C, NB, HEADS).transpose(2, 1, 0).reshape(HEADS, N)
        outs.append(att.reshape(HEADS, HC, N) / cs_t[:, None, :])
    return np.stack(outs, axis=0).reshape(B, C, H, W)
